# revision 23
# baseline (speedup 1.0000x reference)
"""InternLM2 decoder layer on 8 trn2 NeuronCores, tensor-parallel (bass/Tile).

Self-contained: hardcodes shapes/sharding. Host pre-transposes x (xT
replicated to all cores) and pre-tiles weights (bf16, RMSNorm gammas folded
into consuming matmul weights); the device computes the whole layer in
transposed [hid, tok] layout; the host reassembles + transposes the output.

Design notes:
- No AllGather of normalized activations. Every core holds full xT and
  computes the rmsnorm scale row (1/rms per token) itself: sum-of-squares
  via ones-vector matmul (partition reduction), sqrt+eps on ACT, reciprocal
  on DVE, partition broadcast on gpsimd, batched [128,4,CH] DVE multiplies
  (DVE per-op overhead is large; 4-ktile batching keeps DVE off the
  critical path).
- Attention output is produced TRANSPOSED (oT[hid, tok], wo stationary,
  aoT moving) and summed across cores with a per-chunk AllReduce; h and
  norm2 then happen locally on every core in the layout the MLP consumes.
  All DMA uses >=1KB per-partition runs: short strided runs measurably slow
  concurrent matmuls ~20%.
- Softmax: head pairs with score->exp->prob(pv/den) software pipelining
  (pv/den lag one key-tile behind scores) so the PE never waits on the
  scalar engine's exp. Denominator rows for the pair pack at partitions
  0/32 of one PSUM bank. PSUM->SBUF copies run on ACT, not DVE.
- MLP processes token chunks in fused pairs per weight load. After g/u,
  hts is converted in place to h/8 (hhat*rms/8) so the down-proj just adds
  it into its partial outputs; the kb-halved ReduceScatter then directly
  yields the final residual sum, and the tail is a bf16->f32 copy of the
  last RS pieces. norm2 for the last chunk is emitted inside the first MLP
  pair so its AllReduce wait does not block the PE queue.
"""
import sys
import numpy as np
import ml_dtypes

sys.path.insert(0, "/opt/trn_rl_repo")

HID, H, K, D, INTER, T = 4096, 32, 8, 128, 14336, 2048
EPS, THETA = 1e-5, 1000000.0
NC = 8                 # cores
QH = H // NC           # q heads per core = 4
JD = QH * D            # per-core attn out dim = 512
IS = INTER // NC       # inter shard = 1792
CH = 512               # token chunk
NCH = T // CH          # 4
KB_ = HID // 128       # 32 k-tiles
IT_ = IS // 128        # 14 i-tiles
SCALE = 1.0 / np.sqrt(D)

bf16 = ml_dtypes.bfloat16

_compiled = None


def _build():
    from contextlib import ExitStack
    import concourse.bacc as bacc
    import concourse.bass as bass
    import concourse.tile as tile
    from concourse import mybir

    f32 = mybir.dt.float32
    bf = mybir.dt.bfloat16
    AF = mybir.ActivationFunctionType
    PSUM = bass.MemorySpace.PSUM

    nc = bacc.Bacc("TRN2", target_bir_lowering=False, debug=False,
                   num_devices=NC)

    # ---- I/O ----
    xT = nc.dram_tensor("xT", [128, KB_, T], bf, kind="ExternalInput")
    cosT = nc.dram_tensor("cosT", [D // 2, T], bf, kind="ExternalInput")
    sinT = nc.dram_tensor("sinT", [D // 2, T], bf, kind="ExternalInput")
    ident = nc.dram_tensor("ident", [128, 128], bf, kind="ExternalInput")
    masksI = nc.dram_tensor("masksI", [128, 4, CH], bf, kind="ExternalInput")
    wqkvR = nc.dram_tensor("wqkvR", [6, 128, KB_, 128], bf,
                           kind="ExternalInput")
    woR = nc.dram_tensor("woR", [8, 128, QH, 512], bf, kind="ExternalInput")
    w1R = nc.dram_tensor("w1R", [IT_, 128, KB_, 128], bf,
                         kind="ExternalInput")
    w3R = nc.dram_tensor("w3R", [IT_, 128, KB_, 128], bf,
                         kind="ExternalInput")
    w2R = nc.dram_tensor("w2R", [8, 128, IT_, 512], bf, kind="ExternalInput")
    # out_own[q][half][p16][kb16][t]: hid row = (16*half+kb16)*128+16*core+p16
    out_own = nc.dram_tensor("out_own", [NCH, 2, 16, 16, CH], f32,
                             kind="ExternalOutput")

    # ---- internal DRAM ----
    ar_in = [nc.dram_tensor(f"ar_in{j}", [128, KB_, CH], bf, kind="Internal")
             for j in range(NCH)]
    ar_out = [nc.dram_tensor(f"ar_out{j}", [128, KB_, CH], bf,
                             kind="Internal", addr_space="Shared")
              for j in range(NCH)]
    hhat = [nc.dram_tensor(f"hhat{j}", [128, KB_, CH], bf, kind="Internal")
            for j in range(NCH)]
    rms_d = [nc.dram_tensor(f"rms_d{j}", [1, CH], f32, kind="Internal")
             for j in range(NCH)]
    rsm_in = [[nc.dram_tensor(f"rsm_in{q}_{h}", [128, 16, CH], bf,
                              kind="Internal") for h in range(2)]
              for q in range(NCH)]
    rsm_out = [[nc.dram_tensor(f"rsm_out{q}_{h}", [16, 16, CH], bf,
                               kind="Internal")
                for h in range(2)] for q in range(NCH)]

    RG = [list(range(NC))]

    def allreduce(in_t, out_t):
        nc.gpsimd.collective_compute(
            "AllReduce", mybir.AluOpType.add, replica_groups=RG,
            ins=[in_t.ap()], outs=[out_t.ap()])

    def reducescatter(in_t, out_t):
        nc.gpsimd.collective_compute(
            "ReduceScatter", mybir.AluOpType.add, replica_groups=RG,
            ins=[in_t.ap()], outs=[out_t.ap()])

    with tile.TileContext(nc) as tc, ExitStack() as top:
        const = top.enter_context(tc.tile_pool(name="const", bufs=1))
        ones_sb = const.tile([128, 1], bf)
        nc.vector.memset(ones_sb[:], 1.0)
        eps_sb = const.tile([1, 1], f32)
        nc.vector.memset(eps_sb[:], EPS)

        # strip-wise h = xT + attn (ar_out[j]); spill RAW h + rms row;
        # the MLP normalizes chunks on load.
        def norm2(j, dspool, sqpool, rpool, gate=None):
            def _gate(inst):
                if gate is not None:
                    bass._add_dep_helper(
                        inst.ins, gate.ins, sync=False,
                        reason="norm2 scheduled after softmax")
            with nc.named_scope(f"norm2_{j}"):
                ds = dspool.tile([65, CH], f32, tag="ds", name=f"h{j}ds")
                for g in range(8):
                    hs = sqpool.tile([128, 4, CH], bf, tag="xs",
                                     name=f"h{j}hs")
                    _gate(nc.sync.dma_start(
                        hs[:], ar_out[j].ap()[:, 4 * g:4 * g + 4, :]))
                    xs = sqpool.tile([128, 4, CH], bf, tag="sq",
                                     name=f"h{j}xs")
                    _gate(nc.sync.dma_start(
                        xs[:], xT.ap()[:, 4 * g:4 * g + 4,
                                       j * CH:(j + 1) * CH]))
                    nc.vector.tensor_add(hs[:], hs[:], xs[:])
                    nc.sync.dma_start(
                        hhat[j].ap()[:, 4 * g:4 * g + 4, :], hs[:])
                    nc.vector.tensor_mul(xs[:], hs[:], hs[:])
                    for a in range(4):
                        kb = 4 * g + a
                        nc.tensor.matmul(ds[64:65, :], ones_sb[:],
                                         xs[:, a, :], start=(kb == 0),
                                         stop=(kb == KB_ - 1))
                rms = rpool.tile([1, CH], f32, tag="rms", name=f"h{j}rms")
                nc.scalar.activation(rms[:], ds[64:65, :], AF.Sqrt,
                                     scale=1.0 / HID, bias=eps_sb[:])
                nc.sync.dma_start(rms_d[j].ap(), rms[:])

        # ======================= attention phase =======================
        aph = ExitStack()
        aconst = aph.enter_context(tc.tile_pool(name="aconst", bufs=1))
        wqp = aph.enter_context(tc.tile_pool(name="wqkv", bufs=1))
        wop = aph.enter_context(tc.tile_pool(name="wo", bufs=2))
        xpool = aph.enter_context(tc.tile_pool(name="xTp", bufs=2))
        kvp = aph.enter_context(tc.tile_pool(name="kv", bufs=1))
        qa = aph.enter_context(tc.tile_pool(name="qa", bufs=2))
        aop = aph.enter_context(tc.tile_pool(name="aop", bufs=1))
        rp = aph.enter_context(tc.tile_pool(name="ropev", bufs=2))
        ptp = aph.enter_context(tc.tile_pool(name="probs", bufs=5))
        rowp = aph.enter_context(tc.tile_pool(name="rows", bufs=2))
        stg = aph.enter_context(tc.tile_pool(name="stage", bufs=2))
        ps_acc = aph.enter_context(tc.tile_pool(name="accps", bufs=2,
                                                space=PSUM))
        ps_sc = aph.enter_context(tc.tile_pool(name="scps", bufs=3,
                                               space=PSUM))
        ps_pv = aph.enter_context(tc.tile_pool(name="pvps", bufs=2,
                                               space=PSUM))
        ps_ds = aph.enter_context(tc.tile_pool(name="dsps", bufs=1,
                                               space=PSUM))

        ident_sb = aconst.tile([128, 128], bf)
        masks_sb = aconst.tile([128, 4, CH], bf)
        cos_sb = aconst.tile([D // 2, T], bf)
        sin_sb = aconst.tile([D // 2, T], bf)

        kT_sb = kvp.tile([128, T], bf)              # roped K, [d, s]
        v_sb = kvp.tile([128, T // 128, D], bf)     # [s-part, s-tile, d]

        def rope(dst, src, c, s):
            t1 = rp.tile([64, CH], f32, tag="rp1")
            t2 = rp.tile([64, CH], f32, tag="rp2")
            nc.vector.tensor_mul(t1[:], src[0:64, :], c)
            nc.vector.tensor_mul(t2[:], src[64:128, :], s)
            nc.vector.tensor_sub(dst[0:64, :], t1[:], t2[:])
            nc.vector.tensor_mul(t1[:], src[64:128, :], c)
            nc.vector.tensor_mul(t2[:], src[0:64, :], s)
            return nc.vector.tensor_add(dst[64:128, :], t1[:], t2[:])

        def xhat_prep(j, xt=None):
            # raw x chunk; rinv row folded into per-chunk cosR/sinR (rope
            # is linear in cos/sin) and an rcs tile for V.
            if xt is None:
                xt = xpool.tile([128, KB_, CH], bf, tag="xt", name=f"xt{j}")
                for g_ in range(8):
                    nc.sync.dma_start(
                        xt[:, 4 * g_:4 * g_ + 4, :],
                        xT.ap()[:, 4 * g_:4 * g_ + 4,
                                j * CH:(j + 1) * CH])
            ds = ps_ds.tile([65, CH], f32, tag="ds", name=f"x{j}ds")
            for g in range(KB_ // 4):
                sq = stg.tile([128, 4, CH], bf, tag="sq", name=f"x{j}sq")
                nc.vector.tensor_mul(sq[:], xt[:, 4 * g:4 * g + 4, :],
                                     xt[:, 4 * g:4 * g + 4, :])
                for a in range(4):
                    kb = 4 * g + a
                    nc.tensor.matmul(ds[64:65, :], ones_sb[:], sq[:, a, :],
                                     start=(kb == 0), stop=(kb == KB_ - 1))
            rms = rowp.tile([1, CH], f32, tag="rms", name=f"x{j}rms")
            nc.scalar.activation(rms[:], ds[64:65, :], AF.Sqrt,
                                 scale=1.0 / HID, bias=eps_sb[:])
            rin0 = rowp.tile([1, CH], f32, tag="rin0", name=f"x{j}r0")
            nc.vector.reciprocal_approx_fast(rin0[:], rms[:])
            rinv = rowp.tile([1, CH], bf, tag="rinv", name=f"x{j}ri")
            nc.vector.tensor_copy(rinv[:], rin0[:])
            rcs = rowp.tile([128, CH], bf, tag="rcs", name=f"x{j}rcs")
            nc.gpsimd.partition_broadcast(rcs[:], rinv[:])
            t0_ = j * CH
            cosR = rp.tile([64, CH], bf, tag="cosR", name=f"cosR{j}")
            sinR = rp.tile([64, CH], bf, tag="sinR", name=f"sinR{j}")
            nc.vector.tensor_mul(cosR[:], cos_sb[:, t0_:t0_ + CH],
                                 rcs[0:64, :])
            nc.vector.tensor_mul(sinR[:], sin_sb[:, t0_:t0_ + CH],
                                 rcs[0:64, :])
            return xt, cosR, sinR, rcs

        wq_sb = wqp.tile([128, 6, KB_, 128], bf)
        for m in range(6):
            nc.scalar.dma_start(wq_sb[:, m, :, :], wqkvR.ap()[m])

        xt00 = xpool.tile([128, KB_, CH], bf, tag="xt", name="xt0pre")
        for g_ in range(8):
            nc.sync.dma_start(xt00[:, 4 * g_:4 * g_ + 4, :],
                              xT.ap()[:, 4 * g_:4 * g_ + 4, 0:CH])
        nc.sync.dma_start(cos_sb[:], cosT.ap())
        nc.sync.dma_start(sin_sb[:], sinT.ap())
        nc.sync.dma_start(ident_sb[:], ident.ap())
        nc.sync.dma_start(masks_sb[:], masksI.ap())

        xts = [None] * NCH
        xts[0] = xhat_prep(0, xt00)
        for j in range(NCH):
            t0 = j * CH
            ns = 4 * j + 4
            with nc.named_scope(f"attn{j}"):
                xt, cosR, sinR, rcs = xts[j]
                # ---- QKV (resident weights) ----
                qT = qa.tile([128, QH, CH], bf, tag="qT")
                for m in range(6):
                    acc = ps_acc.tile([128, CH], f32, tag="acc", name="acc")
                    for kb in range(KB_):
                        nc.tensor.matmul(acc[:], wq_sb[:, m, kb, :],
                                         xt[:, kb, :],
                                         start=(kb == 0),
                                         stop=(kb == KB_ - 1))
                    if m < QH:
                        rope(qT[:, m, :], acc, cosR[:], sinR[:])
                    elif m == QH:
                        rope(kT_sb[:, t0:t0 + CH], acc, cosR[:], sinR[:])
                    else:
                        vb = rp.tile([128, CH], bf, tag="vb")
                        nc.vector.tensor_mul(vb[:], acc[:], rcs[:])
                        for sb_ in range(CH // 128):
                            tp = ps_sc.tile([128, 128], bf, tag="sc",
                                            name="tp")
                            nc.tensor.transpose(
                                tp[:], vb[:, sb_ * 128:(sb_ + 1) * 128],
                                ident_sb[:])
                            nc.vector.tensor_copy(v_sb[:, 4 * j + sb_, :],
                                                  tp[:])
                # prep next chunk's x while softmax runs
                if j + 1 < NCH:
                    xts[j + 1] = xhat_prep(j + 1)

                # ---- softmax: head pairs, pv/den lag one step ----
                aoT = aop.tile([128, QH, CH], bf, tag="aoT")
                for pr in range(QH // 2):
                    h0, h1 = 2 * pr, 2 * pr + 1
                    ds = ps_ds.tile([65, CH], f32, tag="ds",
                                    name=f"den{j}_{pr}")
                    pvs = {}
                    doff = {h0: 0, h1: 32}
                    for h in (h0, h1):
                        pvs[h] = ps_pv.tile([128, CH], f32, tag="pv",
                                            name=f"pv{h}")

                    def emit_pd(h, si, pT):
                        nc.tensor.matmul(pvs[h][:], v_sb[:, si, :], pT[:],
                                         start=(si == 0),
                                         stop=(si == ns - 1))
                        o = doff[h]
                        nc.tensor.matmul(ds[o:o + 1, :], ones_sb[:], pT[:],
                                         start=(si == 0),
                                         stop=(si == ns - 1))

                    pend = []
                    for si in range(ns):
                        for h in (h0, h1):
                            sc = ps_sc.tile([128, CH], f32, tag="sc",
                                            name="sc")
                            nc.tensor.matmul(
                                sc[:], kT_sb[:, si * 128:(si + 1) * 128],
                                qT[:, h, :], start=True, stop=True)
                            pT = ptp.tile([128, CH], bf, tag="pT", name="pT")
                            last_exp = nc.scalar.activation(
                                pT[:], sc[:], AF.Exp, scale=SCALE)
                            if si >= 4 * j:      # diagonal: zero s > t
                                pm = ptp.tile([128, CH], bf, tag="pm",
                                              name="pm")
                                nc.vector.tensor_mul(
                                    pm[:], pT[:],
                                    masks_sb[:, si - 4 * j, :])
                                pT = pm
                            pend.append((h, si, pT))
                        while len(pend) > 4:     # pv/den lag two key-tiles
                            emit_pd(*pend.pop(0))
                    for tup in pend:
                        emit_pd(*tup)
                    for h in (h0, h1):
                        o = doff[h]
                        rr0 = rowp.tile([1, CH], f32, tag="rin0",
                                        name="rr0")
                        nc.vector.reciprocal(rr0[:], ds[o:o + 1, :])
                        rr = rowp.tile([1, CH], bf, tag="rinv", name="rr")
                        nc.vector.tensor_copy(rr[:], rr0[:])
                        rcs = rowp.tile([128, CH], bf, tag="rcs",
                                        name="rcs")
                        nc.gpsimd.partition_broadcast(rcs[:], rr[:])
                        nc.vector.tensor_mul(aoT[:, h, :], pvs[h][:],
                                             rcs[:])

                # ---- wo -> oT[hid, tok] partials -> AllReduce ----
                for g in range(8):
                    woS = wop.tile([128, QH, 512], bf, tag="woS", name="woS")
                    nc.scalar.dma_start(woS[:], woR.ap()[g])
                    for hb4 in range(4):
                        acc = ps_acc.tile([128, CH], f32, tag="acc",
                                          name="woacc")
                        for kb in range(QH):
                            nc.tensor.matmul(
                                acc[:],
                                woS[:, kb, hb4 * 128:(hb4 + 1) * 128],
                                aoT[:, kb, :],
                                start=(kb == 0), stop=(kb == QH - 1))
                        ob = stg.tile([128, CH], bf, tag="ob", name="ob")
                        nc.vector.tensor_copy(ob[:], acc[:])
                        nc.sync.dma_start(
                            ar_in[j].ap()[:, g * 4 + hb4, :], ob[:])
                allreduce(ar_in[j], ar_out[j])
                # previous chunk's norm2 AFTER this chunk's softmax/wo so a
                # slow AllReduce can never head-of-line-block the PE queue
                if j >= 1:
                    norm2(j - 1, ps_ds, stg, rowp, gate=last_exp)

        aph.close()

        # ========================= MLP phase =========================
        mph = ExitStack()
        hpool = mph.enter_context(tc.tile_pool(name="hh", bufs=2))
        actp = mph.enter_context(tc.tile_pool(name="act", bufs=1))
        mw = mph.enter_context(tc.tile_pool(name="mw", bufs=2))
        mw2 = mph.enter_context(tc.tile_pool(name="mw2", bufs=2))
        mstg = mph.enter_context(tc.tile_pool(name="mstage", bufs=2))
        rowp2 = mph.enter_context(tc.tile_pool(name="rows2", bufs=1))
        roww2 = mph.enter_context(tc.tile_pool(name="roww2", bufs=2))
        ps_g = mph.enter_context(tc.tile_pool(name="gups", bufs=2,
                                              space=PSUM))
        ps_d = mph.enter_context(tc.tile_pool(name="dps", bufs=2,
                                              space=PSUM))
        ps_ds2 = mph.enter_context(tc.tile_pool(name="dsps2", bufs=1,
                                                space=PSUM))

        for P in ((0, 1), (2, 3)):
            with nc.named_scope(f"mlp{P[0]}{P[1]}"):
                hts, r8w, w2pre = {}, {}, {}
                for q in P:
                    hts[q] = hpool.tile([128, KB_, CH], bf, tag="hh",
                                        name=f"hh{q}")
                    nc.sync.dma_start(hts[q][:], hhat[q].ap())
                    rr3 = rowp2.tile([1, CH], f32, tag="rms",
                                     name=f"rr3{q}")
                    nc.sync.dma_start(rr3[:], rms_d[q].ap())
                    ri30 = rowp2.tile([1, CH], f32, tag="rin0",
                                      name=f"ri30{q}")
                    nc.vector.reciprocal_approx_fast(ri30[:], rr3[:])
                    ri3 = rowp2.tile([1, CH], bf, tag="rinv",
                                     name=f"ri3{q}")
                    nc.vector.tensor_copy(ri3[:], ri30[:])
                    rw3 = roww2.tile([128, 4, CH], bf, tag="rcsw",
                                     name=f"rw3{q}")
                    for a in range(4):
                        nc.gpsimd.partition_broadcast(rw3[:, a, :],
                                                      ri3[:])
                    for g in range(KB_ // 4):
                        nc.vector.tensor_mul(
                            hts[q][:, 4 * g:4 * g + 4, :],
                            hts[q][:, 4 * g:4 * g + 4, :], rw3[:])
                acts = {q: actp.tile([128, IT_, CH], bf, tag=f"act{qi}",
                                     name=f"act{q}")
                        for qi, q in enumerate(P)}
                for it in range(IT_):
                    w1t = mw.tile([128, KB_, 128], bf, tag="w1t",
                                  name="w1t")
                    w3t = mw.tile([128, KB_, 128], bf, tag="w3t",
                                  name="w3t")
                    nc.scalar.dma_start(w1t[:], w1R.ap()[it])
                    nc.scalar.dma_start(w3t[:], w3R.ap()[it])
                    for q in P:
                        gp = ps_g.tile([128, CH], f32, tag="g", name="g")
                        up = ps_g.tile([128, CH], f32, tag="u", name="u")
                        for kb in range(KB_):
                            nc.tensor.matmul(gp[:], w1t[:, kb, :],
                                             hts[q][:, kb, :],
                                             start=(kb == 0),
                                             stop=(kb == KB_ - 1))
                        for kb in range(KB_):
                            nc.tensor.matmul(up[:], w3t[:, kb, :],
                                             hts[q][:, kb, :],
                                             start=(kb == 0),
                                             stop=(kb == KB_ - 1))
                        sg = mstg.tile([128, CH], bf, tag="sg", name="sg")
                        nc.scalar.activation(sg[:], gp[:], AF.Silu)
                        last_mul = nc.vector.tensor_mul(acts[q][:, it, :],
                                                        sg[:], up[:])
                    if it >= IT_ - 2:     # prefetch first down-proj weights
                        s8p = it - (IT_ - 2)
                        w2pre[s8p] = mw2.tile([128, IT_, 512], bf,
                                              tag="w2s", name="w2s")
                        nc.scalar.dma_start(w2pre[s8p][:], w2R.ap()[s8p])
                    # last-chunk norm2 sits here so its AllReduce wait
                    # cannot block the PE queue at MLP entry
                    if P[0] == 0 and it == 6:
                        norm2(NCH - 1, ps_ds2, mstg, rowp2,
                              gate=last_mul)

                # convert hts in place to h/8 = hhat * rms/8 for the
                # residual fold (g/u no longer needs the normalized value)
                for q in P:
                    rr2 = rowp2.tile([1, CH], f32, tag="rms",
                                     name=f"rr2{q}")
                    nc.sync.dma_start(rr2[:], rms_d[q].ap())
                    r8 = rowp2.tile([1, CH], bf, tag="rinv", name=f"r8{q}")
                    nc.vector.tensor_scalar_mul(r8[:], rr2[:], 1.0 / NC)
                    r8w[q] = roww2.tile([128, 4, CH], bf, tag="rcsw",
                                        name=f"r8w{q}")
                    for a in range(4):
                        nc.gpsimd.partition_broadcast(r8w[q][:, a, :],
                                                      r8[:])
                    for g in range(KB_ // 4):
                        nc.vector.tensor_mul(hts[q][:, 4 * g:4 * g + 4, :],
                                             hts[q][:, 4 * g:4 * g + 4, :],
                                             r8w[q][:])

                # down-proj; h/8 folded into the RS inputs
                for s8 in range(8):
                    if s8 in w2pre:
                        w2s = w2pre.pop(s8)
                    else:
                        w2s = mw2.tile([128, IT_, 512], bf, tag="w2s",
                                       name="w2s")
                        nc.scalar.dma_start(w2s[:], w2R.ap()[s8])
                    for q in reversed(P):
                        for hb4 in range(4):
                            hb = s8 * 4 + hb4
                            accd = ps_d.tile([128, CH], f32, tag="d",
                                             name="d")
                            for it in range(IT_):
                                nc.tensor.matmul(
                                    accd[:],
                                    w2s[:, it, hb4 * 128:(hb4 + 1) * 128],
                                    acts[q][:, it, :],
                                    start=(it == 0), stop=(it == IT_ - 1))
                            ob = mstg.tile([128, CH], bf, tag="ob",
                                           name="ob")
                            nc.vector.tensor_add(ob[:], accd[:],
                                                 hts[q][:, hb, :])
                            nc.sync.dma_start(
                                rsm_in[q][hb // 16].ap()[:, hb % 16, :],
                                ob[:])
                    if s8 == 3:
                        for q in reversed(P):
                            reducescatter(rsm_in[q][0], rsm_out[q][0])
                for q in reversed(P):
                    reducescatter(rsm_in[q][1], rsm_out[q][1])

        mph.close()

        # ===== final: out = f32(rsm_out) (h folded via h/8 trick) =====
        with ExitStack() as fph, nc.named_scope("fin"):
            finp = fph.enter_context(tc.tile_pool(name="finp", bufs=2))
            for q in (0, 1, 3, 2):
                for half in range(2):
                    rt = finp.tile([128, 2, CH], bf, tag="frt", name="frt")
                    nc.sync.dma_start(rt[:], rsm_out[q][half].ap())
                    ot = finp.tile([128, 2, CH], f32, tag="fot", name="fot")
                    nc.vector.tensor_copy(ot[:], rt[:])
                    nc.sync.dma_start(out_own.ap()[q][half], ot[:])

    nc.compile()
    return nc


def _get_compiled():
    global _compiled
    if _compiled is None:
        _compiled = _build()
    return _compiled


def _prep_inputs(inputs):
    x = np.asarray(inputs["hidden_states"], np.float32)
    pos = np.asarray(inputs["position_ids"]).astype(np.float32)
    wqkv = np.asarray(inputs["wqkv"], np.float32)
    wo = np.asarray(inputs["wo"], np.float32)
    w1 = np.asarray(inputs["w1"], np.float32)
    w3 = np.asarray(inputs["w3"], np.float32)
    w2 = np.asarray(inputs["w2"], np.float32)
    anw = np.asarray(inputs["attn_norm_w"], np.float32)
    fnw = np.asarray(inputs["ffn_norm_w"], np.float32)

    inv_freq = 1.0 / (THETA ** (np.arange(0, D, 2, dtype=np.float32) / D))
    freqs = pos[:, None] * inv_freq
    cosT_np = np.ascontiguousarray(np.cos(freqs).T.astype(bf16))
    sinT_np = np.ascontiguousarray(np.sin(freqs).T.astype(bf16))
    ident_np = np.ascontiguousarray(np.eye(128, dtype=bf16))

    # causal masks for diagonal tiles: masks[p, r, f] = (f >= 128*r + p)
    p_ = np.arange(128)[:, None, None]
    r_ = np.arange(4)[None, :, None]
    f_ = np.arange(CH)[None, None, :]
    masks_np = np.ascontiguousarray((f_ >= 128 * r_ + p_).astype(bf16))

    xT_np = np.ascontiguousarray(
        x.T.reshape(KB_, 128, T).transpose(1, 0, 2).astype(bf16))

    wqkv_f = wqkv * anw[None, :]
    w1_f = w1 * fnw[None, :]
    w3_f = w3 * fnw[None, :]

    in_maps = []
    for c in range(NC):
        qrows = np.arange(JD * c, JD * (c + 1))
        krows = H * D + np.arange(D * c, D * (c + 1))
        vrows = (H + K) * D + np.arange(D * c, D * (c + 1))
        rows = np.concatenate([qrows, krows, vrows])
        wqkvT = wqkv_f[rows].T                      # [HID, 768]
        w1T = w1_f[IS * c:IS * (c + 1)].T           # [HID, IS]
        w3T = w3_f[IS * c:IS * (c + 1)].T
        in_maps.append({
            "xT": xT_np,
            "cosT": cosT_np, "sinT": sinT_np, "ident": ident_np,
            "masksI": masks_np,
            "wqkvR": np.ascontiguousarray(
                wqkvT.reshape(KB_, 128, 6, 128).transpose(2, 1, 0, 3)
                .astype(bf16)),
            "woR": np.ascontiguousarray(
                wo[:, JD * c:JD * (c + 1)].T.reshape(QH, 128, 8, 512)
                .transpose(2, 1, 0, 3).astype(bf16)),
            "w1R": np.ascontiguousarray(
                w1T.reshape(KB_, 128, IT_, 128).transpose(2, 1, 0, 3)
                .astype(bf16)),
            "w3R": np.ascontiguousarray(
                w3T.reshape(KB_, 128, IT_, 128).transpose(2, 1, 0, 3)
                .astype(bf16)),
            "w2R": np.ascontiguousarray(
                w2[:, IS * c:IS * (c + 1)].T.reshape(IT_, 128, 8, 512)
                .transpose(2, 1, 0, 3).astype(bf16)),
        })
    return in_maps


def run(inputs, trace=False):
    """Returns (output, BassKernelResults)."""
    from concourse import bass_utils
    nc = _get_compiled()
    in_maps = _prep_inputs(inputs)
    res = bass_utils.run_bass_kernel_spmd(
        nc, in_maps, core_ids=list(range(NC)), trace=trace)
    # out_own[c][q, half, p16, kb16, t]: outT[(16*half+kb16)*128+16*c+p16,
    #                                        512*q+t]
    arr = np.stack([res.results[c]["out_own"] for c in range(NC)])
    outT = arr.transpose(2, 4, 0, 3, 1, 5).reshape(HID, T)
    return np.ascontiguousarray(outT.T), res


def kernel(**inputs):
    out, _ = run(inputs)
    return out


# revision 24
# speedup vs baseline: 1.0208x; 1.0208x over previous
"""InternLM2 decoder layer on 8 trn2 NeuronCores, tensor-parallel (bass/Tile).

Self-contained: hardcodes shapes/sharding. Host pre-transposes x (xT
replicated to all cores) and pre-tiles weights (bf16, RMSNorm gammas folded
into consuming matmul weights); the device computes the whole layer in
transposed [hid, tok] layout; the host reassembles + transposes the output.

Design notes:
- No AllGather of normalized activations. Every core holds full xT and
  computes the rmsnorm scale row (1/rms per token) itself: sum-of-squares
  via ones-vector matmul (partition reduction), sqrt+eps on ACT, reciprocal
  on DVE, partition broadcast on gpsimd, batched [128,4,CH] DVE multiplies
  (DVE per-op overhead is large; 4-ktile batching keeps DVE off the
  critical path).
- Attention output is produced TRANSPOSED (oT[hid, tok], wo stationary,
  aoT moving) and summed across cores with a per-chunk AllReduce; h and
  norm2 then happen locally on every core in the layout the MLP consumes.
  All DMA uses >=1KB per-partition runs: short strided runs measurably slow
  concurrent matmuls ~20%.
- Softmax: head pairs with score->exp->prob(pv/den) software pipelining
  (pv/den lag one key-tile behind scores) so the PE never waits on the
  scalar engine's exp. Denominator rows for the pair pack at partitions
  0/32 of one PSUM bank. PSUM->SBUF copies run on ACT, not DVE.
- MLP processes token chunks in fused pairs per weight load. After g/u,
  hts is converted in place to h/8 (hhat*rms/8) so the down-proj just adds
  it into its partial outputs; the kb-halved ReduceScatter then directly
  yields the final residual sum, and the tail is a bf16->f32 copy of the
  last RS pieces. norm2 for the last chunk is emitted inside the first MLP
  pair so its AllReduce wait does not block the PE queue.
"""
import sys
import numpy as np
import ml_dtypes

sys.path.insert(0, "/opt/trn_rl_repo")

HID, H, K, D, INTER, T = 4096, 32, 8, 128, 14336, 2048
EPS, THETA = 1e-5, 1000000.0
NC = 8                 # cores
QH = H // NC           # q heads per core = 4
JD = QH * D            # per-core attn out dim = 512
IS = INTER // NC       # inter shard = 1792
CH = 512               # token chunk
NCH = T // CH          # 4
KB_ = HID // 128       # 32 k-tiles
IT_ = IS // 128        # 14 i-tiles
SCALE = 1.0 / np.sqrt(D)

bf16 = ml_dtypes.bfloat16

_compiled = None


def _build():
    from contextlib import ExitStack
    import concourse.bacc as bacc
    import concourse.bass as bass
    import concourse.tile as tile
    from concourse import mybir

    f32 = mybir.dt.float32
    bf = mybir.dt.bfloat16
    AF = mybir.ActivationFunctionType
    PSUM = bass.MemorySpace.PSUM

    nc = bacc.Bacc("TRN2", target_bir_lowering=False, debug=False,
                   num_devices=NC)

    # ---- I/O ----
    xT = nc.dram_tensor("xT", [128, KB_, T], bf, kind="ExternalInput")
    cosT = nc.dram_tensor("cosT", [D // 2, T], bf, kind="ExternalInput")
    sinT = nc.dram_tensor("sinT", [D // 2, T], bf, kind="ExternalInput")
    ident = nc.dram_tensor("ident", [128, 128], bf, kind="ExternalInput")
    masksI = nc.dram_tensor("masksI", [128, 4, CH], bf, kind="ExternalInput")
    wqkvR = nc.dram_tensor("wqkvR", [6, 128, KB_, 128], bf,
                           kind="ExternalInput")
    woR = nc.dram_tensor("woR", [8, 128, QH, 512], bf, kind="ExternalInput")
    w1R = nc.dram_tensor("w1R", [IT_, 128, KB_, 128], bf,
                         kind="ExternalInput")
    w3R = nc.dram_tensor("w3R", [IT_, 128, KB_, 128], bf,
                         kind="ExternalInput")
    w2R = nc.dram_tensor("w2R", [8, 128, IT_, 512], bf, kind="ExternalInput")
    # out_own[q][half][p16][kb16][t]: hid row = (16*half+kb16)*128+16*core+p16
    out_own = nc.dram_tensor("out_own", [NCH, 2, 16, 16, CH], f32,
                             kind="ExternalOutput")

    # ---- internal DRAM ----
    ar_in = [nc.dram_tensor(f"ar_in{j}", [128, KB_, CH], bf, kind="Internal")
             for j in range(NCH)]
    ar_out = [nc.dram_tensor(f"ar_out{j}", [128, KB_, CH], bf,
                             kind="Internal", addr_space="Shared")
              for j in range(NCH)]
    hhat = [nc.dram_tensor(f"hhat{j}", [128, KB_, CH], bf, kind="Internal")
            for j in range(NCH)]
    rms_d = [nc.dram_tensor(f"rms_d{j}", [1, CH], f32, kind="Internal")
             for j in range(NCH)]
    rsm_in = [[nc.dram_tensor(f"rsm_in{q}_{h}", [128, 16, CH], bf,
                              kind="Internal") for h in range(2)]
              for q in range(NCH)]
    rsm_out = [[nc.dram_tensor(f"rsm_out{q}_{h}", [16, 16, CH], bf,
                               kind="Internal")
                for h in range(2)] for q in range(NCH)]

    RG = [list(range(NC))]

    def allreduce(in_t, out_t):
        nc.gpsimd.collective_compute(
            "AllReduce", mybir.AluOpType.add, replica_groups=RG,
            ins=[in_t.ap()], outs=[out_t.ap()])

    def reducescatter(in_t, out_t):
        nc.gpsimd.collective_compute(
            "ReduceScatter", mybir.AluOpType.add, replica_groups=RG,
            ins=[in_t.ap()], outs=[out_t.ap()])

    with tile.TileContext(nc) as tc, ExitStack() as top:
        const = top.enter_context(tc.tile_pool(name="const", bufs=1))
        ones_sb = const.tile([128, 1], bf)
        nc.vector.memset(ones_sb[:], 1.0)
        eps_sb = const.tile([1, 1], f32)
        nc.vector.memset(eps_sb[:], EPS)

        # strip-wise h = xT + attn (ar_out[j]); spill RAW h + rms row;
        # the MLP normalizes chunks on load.
        def norm2(j, dspool, sqpool, rpool, gate=None):
            def _gate(inst):
                if gate is not None:
                    bass._add_dep_helper(
                        inst.ins, gate.ins, sync=False,
                        reason="norm2 scheduled after softmax")
            with nc.named_scope(f"norm2_{j}"):
                ds = dspool.tile([65, CH], f32, tag="ds", name=f"h{j}ds")
                for g in range(8):
                    hs = sqpool.tile([128, 4, CH], bf, tag="xs",
                                     name=f"h{j}hs")
                    _gate(nc.sync.dma_start(
                        hs[:], ar_out[j].ap()[:, 4 * g:4 * g + 4, :]))
                    xs = sqpool.tile([128, 4, CH], bf, tag="sq",
                                     name=f"h{j}xs")
                    _gate(nc.sync.dma_start(
                        xs[:], xT.ap()[:, 4 * g:4 * g + 4,
                                       j * CH:(j + 1) * CH]))
                    nc.vector.tensor_add(hs[:], hs[:], xs[:])
                    nc.sync.dma_start(
                        hhat[j].ap()[:, 4 * g:4 * g + 4, :], hs[:])
                    nc.vector.tensor_mul(xs[:], hs[:], hs[:])
                    for a in range(4):
                        kb = 4 * g + a
                        nc.tensor.matmul(ds[64:65, :], ones_sb[:],
                                         xs[:, a, :], start=(kb == 0),
                                         stop=(kb == KB_ - 1))
                rms = rpool.tile([1, CH], f32, tag="rms", name=f"h{j}rms")
                nc.scalar.activation(rms[:], ds[64:65, :], AF.Sqrt,
                                     scale=1.0 / HID, bias=eps_sb[:])
                nc.sync.dma_start(rms_d[j].ap(), rms[:])

        # ======================= attention phase =======================
        aph = ExitStack()
        aconst = aph.enter_context(tc.tile_pool(name="aconst", bufs=1))
        wqp = aph.enter_context(tc.tile_pool(name="wqkv", bufs=1))
        wop = aph.enter_context(tc.tile_pool(name="wo", bufs=2))
        xpool = aph.enter_context(tc.tile_pool(name="xTp", bufs=2))
        kvp = aph.enter_context(tc.tile_pool(name="kv", bufs=1))
        qa = aph.enter_context(tc.tile_pool(name="qa", bufs=2))
        aop = aph.enter_context(tc.tile_pool(name="aop", bufs=1))
        rp = aph.enter_context(tc.tile_pool(name="ropev", bufs=2))
        ptp = aph.enter_context(tc.tile_pool(name="probs", bufs=5))
        rowp = aph.enter_context(tc.tile_pool(name="rows", bufs=2))
        stg = aph.enter_context(tc.tile_pool(name="stage", bufs=2))
        ps_acc = aph.enter_context(tc.tile_pool(name="accps", bufs=2,
                                                space=PSUM))
        ps_sc = aph.enter_context(tc.tile_pool(name="scps", bufs=3,
                                               space=PSUM))
        ps_pv = aph.enter_context(tc.tile_pool(name="pvps", bufs=2,
                                               space=PSUM))
        ps_ds = aph.enter_context(tc.tile_pool(name="dsps", bufs=1,
                                               space=PSUM))

        ident_sb = aconst.tile([128, 128], bf)
        masks_sb = aconst.tile([128, 4, CH], bf)
        cos_sb = aconst.tile([D // 2, T], bf)
        sin_sb = aconst.tile([D // 2, T], bf)

        kT_sb = kvp.tile([128, T], bf)              # roped K, [d, s]
        v_sb = kvp.tile([128, T // 128, D], bf)     # [s-part, s-tile, d]

        def rope(dst, src, c, s):
            t1 = rp.tile([64, CH], f32, tag="rp1")
            t2 = rp.tile([64, CH], f32, tag="rp2")
            nc.vector.tensor_mul(t1[:], src[0:64, :], c)
            nc.vector.tensor_mul(t2[:], src[64:128, :], s)
            nc.vector.tensor_sub(dst[0:64, :], t1[:], t2[:])
            nc.vector.tensor_mul(t1[:], src[64:128, :], c)
            nc.vector.tensor_mul(t2[:], src[0:64, :], s)
            return nc.vector.tensor_add(dst[64:128, :], t1[:], t2[:])

        def xhat_prep(j, xt=None):
            # raw x chunk; rinv row folded into per-chunk cosR/sinR (rope
            # is linear in cos/sin) and an rcs tile for V.
            if xt is None:
                xt = xpool.tile([128, KB_, CH], bf, tag="xt", name=f"xt{j}")
                for g_ in range(8):
                    nc.sync.dma_start(
                        xt[:, 4 * g_:4 * g_ + 4, :],
                        xT.ap()[:, 4 * g_:4 * g_ + 4,
                                j * CH:(j + 1) * CH])
            ds = ps_ds.tile([65, CH], f32, tag="ds", name=f"x{j}ds")
            for g in range(KB_ // 4):
                sq = stg.tile([128, 4, CH], bf, tag="sq", name=f"x{j}sq")
                nc.vector.tensor_mul(sq[:], xt[:, 4 * g:4 * g + 4, :],
                                     xt[:, 4 * g:4 * g + 4, :])
                for a in range(4):
                    kb = 4 * g + a
                    nc.tensor.matmul(ds[64:65, :], ones_sb[:], sq[:, a, :],
                                     start=(kb == 0), stop=(kb == KB_ - 1))
            rms = rowp.tile([1, CH], f32, tag="rms", name=f"x{j}rms")
            nc.scalar.activation(rms[:], ds[64:65, :], AF.Sqrt,
                                 scale=1.0 / HID, bias=eps_sb[:])
            rin0 = rowp.tile([1, CH], f32, tag="rin0", name=f"x{j}r0")
            nc.vector.reciprocal_approx_fast(rin0[:], rms[:])
            rinv = rowp.tile([1, CH], bf, tag="rinv", name=f"x{j}ri")
            nc.vector.tensor_copy(rinv[:], rin0[:])
            rcs = rowp.tile([128, CH], bf, tag="rcs", name=f"x{j}rcs")
            nc.gpsimd.partition_broadcast(rcs[:], rinv[:])
            t0_ = j * CH
            cosR = rp.tile([64, CH], bf, tag="cosR", name=f"cosR{j}")
            sinR = rp.tile([64, CH], bf, tag="sinR", name=f"sinR{j}")
            nc.vector.tensor_mul(cosR[:], cos_sb[:, t0_:t0_ + CH],
                                 rcs[0:64, :])
            nc.vector.tensor_mul(sinR[:], sin_sb[:, t0_:t0_ + CH],
                                 rcs[0:64, :])
            return xt, cosR, sinR, rcs

        wq_sb = wqp.tile([128, 6, KB_, 128], bf)
        for m in range(6):
            nc.scalar.dma_start(wq_sb[:, m, :, :], wqkvR.ap()[m])

        xt00 = xpool.tile([128, KB_, CH], bf, tag="xt", name="xt0pre")
        for g_ in range(8):
            nc.sync.dma_start(xt00[:, 4 * g_:4 * g_ + 4, :],
                              xT.ap()[:, 4 * g_:4 * g_ + 4, 0:CH])
        nc.sync.dma_start(cos_sb[:], cosT.ap())
        nc.sync.dma_start(sin_sb[:], sinT.ap())
        nc.sync.dma_start(ident_sb[:], ident.ap())
        nc.sync.dma_start(masks_sb[:], masksI.ap())

        xts = [None] * NCH
        xts[0] = xhat_prep(0, xt00)
        for j in range(NCH):
            t0 = j * CH
            ns = 4 * j + 4
            with nc.named_scope(f"attn{j}"):
                xt, cosR, sinR, rcs = xts[j]
                # ---- QKV (resident weights) ----
                qT = qa.tile([128, QH, CH], bf, tag="qT")
                for m in range(6):
                    acc = ps_acc.tile([128, CH], f32, tag="acc", name="acc")
                    for kb in range(KB_):
                        nc.tensor.matmul(acc[:], wq_sb[:, m, kb, :],
                                         xt[:, kb, :],
                                         start=(kb == 0),
                                         stop=(kb == KB_ - 1))
                    if m < QH:
                        rope(qT[:, m, :], acc, cosR[:], sinR[:])
                    elif m == QH:
                        rope(kT_sb[:, t0:t0 + CH], acc, cosR[:], sinR[:])
                    else:
                        vb = rp.tile([128, CH], bf, tag="vb")
                        nc.vector.tensor_mul(vb[:], acc[:], rcs[:])
                        for sb_ in range(CH // 128):
                            tp = ps_sc.tile([128, 128], bf, tag="sc",
                                            name="tp")
                            nc.tensor.transpose(
                                tp[:], vb[:, sb_ * 128:(sb_ + 1) * 128],
                                ident_sb[:])
                            nc.vector.tensor_copy(v_sb[:, 4 * j + sb_, :],
                                                  tp[:])
                # prep next chunk's x while softmax runs
                if j + 1 < NCH:
                    xts[j + 1] = xhat_prep(j + 1)

                # ---- softmax: head pairs, pv/den lag one step ----
                aoT = aop.tile([128, QH, CH], bf, tag="aoT")
                for pr in range(QH // 2):
                    h0, h1 = 2 * pr, 2 * pr + 1
                    ds = ps_ds.tile([65, CH], f32, tag="ds",
                                    name=f"den{j}_{pr}")
                    pvs = {}
                    doff = {h0: 0, h1: 32}
                    for h in (h0, h1):
                        pvs[h] = ps_pv.tile([128, CH], f32, tag="pv",
                                            name=f"pv{h}")

                    def emit_pd(h, si, pT):
                        nc.tensor.matmul(pvs[h][:], v_sb[:, si, :], pT[:],
                                         start=(si == 0),
                                         stop=(si == ns - 1))
                        o = doff[h]
                        nc.tensor.matmul(ds[o:o + 1, :], ones_sb[:], pT[:],
                                         start=(si == 0),
                                         stop=(si == ns - 1))

                    pend = []
                    for si in range(ns):
                        for h in (h0, h1):
                            sc = ps_sc.tile([128, CH], f32, tag="sc",
                                            name="sc")
                            nc.tensor.matmul(
                                sc[:], kT_sb[:, si * 128:(si + 1) * 128],
                                qT[:, h, :], start=True, stop=True)
                            pT = ptp.tile([128, CH], bf, tag="pT", name="pT")
                            last_exp = nc.scalar.activation(
                                pT[:], sc[:], AF.Exp, scale=SCALE)
                            if si >= 4 * j:      # diagonal: zero s > t
                                pm = ptp.tile([128, CH], bf, tag="pm",
                                              name="pm")
                                nc.vector.tensor_mul(
                                    pm[:], pT[:],
                                    masks_sb[:, si - 4 * j, :])
                                pT = pm
                            pend.append((h, si, pT))
                        while len(pend) > 4:     # pv/den lag two key-tiles
                            emit_pd(*pend.pop(0))
                    for tup in pend:
                        emit_pd(*tup)
                    for h in (h0, h1):
                        o = doff[h]
                        dsb = rowp.tile([1, CH], f32, tag="rms",
                                        name="dsb")
                        nc.vector.tensor_copy(dsb[:], ds[o:o + 1, :])
                        rr0 = rowp.tile([1, CH], f32, tag="rin0",
                                        name="rr0")
                        nc.vector.reciprocal_approx_fast(rr0[:], dsb[:])
                        rr = rowp.tile([1, CH], bf, tag="rinv", name="rr")
                        nc.vector.tensor_copy(rr[:], rr0[:])
                        rcs = rowp.tile([128, CH], bf, tag="rcs",
                                        name="rcs")
                        nc.gpsimd.partition_broadcast(rcs[:], rr[:])
                        nc.vector.tensor_mul(aoT[:, h, :], pvs[h][:],
                                             rcs[:])

                # ---- wo -> oT[hid, tok] partials -> AllReduce ----
                for g in range(8):
                    woS = wop.tile([128, QH, 512], bf, tag="woS", name="woS")
                    nc.scalar.dma_start(woS[:], woR.ap()[g])
                    for hb4 in range(4):
                        acc = ps_acc.tile([128, CH], f32, tag="acc",
                                          name="woacc")
                        for kb in range(QH):
                            nc.tensor.matmul(
                                acc[:],
                                woS[:, kb, hb4 * 128:(hb4 + 1) * 128],
                                aoT[:, kb, :],
                                start=(kb == 0), stop=(kb == QH - 1))
                        ob = stg.tile([128, CH], bf, tag="ob", name="ob")
                        nc.vector.tensor_copy(ob[:], acc[:])
                        nc.sync.dma_start(
                            ar_in[j].ap()[:, g * 4 + hb4, :], ob[:])
                allreduce(ar_in[j], ar_out[j])
                # previous chunk's norm2 AFTER this chunk's softmax/wo so a
                # slow AllReduce can never head-of-line-block the PE queue
                if j >= 1:
                    norm2(j - 1, ps_ds, stg, rowp, gate=last_exp)

        aph.close()

        # ========================= MLP phase =========================
        mph = ExitStack()
        hpool = mph.enter_context(tc.tile_pool(name="hh", bufs=2))
        actp = mph.enter_context(tc.tile_pool(name="act", bufs=1))
        mw = mph.enter_context(tc.tile_pool(name="mw", bufs=2))
        mw2 = mph.enter_context(tc.tile_pool(name="mw2", bufs=2))
        mstg = mph.enter_context(tc.tile_pool(name="mstage", bufs=2))
        rowp2 = mph.enter_context(tc.tile_pool(name="rows2", bufs=1))
        roww2 = mph.enter_context(tc.tile_pool(name="roww2", bufs=2))
        ps_g = mph.enter_context(tc.tile_pool(name="gups", bufs=2,
                                              space=PSUM))
        ps_d = mph.enter_context(tc.tile_pool(name="dps", bufs=2,
                                              space=PSUM))
        ps_ds2 = mph.enter_context(tc.tile_pool(name="dsps2", bufs=1,
                                                space=PSUM))

        for P in ((0, 1), (2, 3)):
            with nc.named_scope(f"mlp{P[0]}{P[1]}"):
                hts, r8w, w2pre = {}, {}, {}
                for q in P:
                    hts[q] = hpool.tile([128, KB_, CH], bf, tag="hh",
                                        name=f"hh{q}")
                    nc.sync.dma_start(hts[q][:], hhat[q].ap())
                    rr3 = rowp2.tile([1, CH], f32, tag="rms",
                                     name=f"rr3{q}")
                    nc.sync.dma_start(rr3[:], rms_d[q].ap())
                    ri30 = rowp2.tile([1, CH], f32, tag="rin0",
                                      name=f"ri30{q}")
                    nc.vector.reciprocal_approx_fast(ri30[:], rr3[:])
                    ri3 = rowp2.tile([1, CH], bf, tag="rinv",
                                     name=f"ri3{q}")
                    nc.vector.tensor_copy(ri3[:], ri30[:])
                    rw3 = roww2.tile([128, 4, CH], bf, tag="rcsw",
                                     name=f"rw3{q}")
                    for a in range(4):
                        nc.gpsimd.partition_broadcast(rw3[:, a, :],
                                                      ri3[:])
                    for g in range(KB_ // 4):
                        nc.vector.tensor_mul(
                            hts[q][:, 4 * g:4 * g + 4, :],
                            hts[q][:, 4 * g:4 * g + 4, :], rw3[:])
                acts = {q: actp.tile([128, IT_, CH], bf, tag=f"act{qi}",
                                     name=f"act{q}")
                        for qi, q in enumerate(P)}
                for it in range(IT_):
                    w1t = mw.tile([128, KB_, 128], bf, tag="w1t",
                                  name="w1t")
                    w3t = mw.tile([128, KB_, 128], bf, tag="w3t",
                                  name="w3t")
                    nc.scalar.dma_start(w1t[:], w1R.ap()[it])
                    nc.scalar.dma_start(w3t[:], w3R.ap()[it])
                    for q in P:
                        gp = ps_g.tile([128, CH], f32, tag="g", name="g")
                        up = ps_g.tile([128, CH], f32, tag="u", name="u")
                        for kb in range(KB_):
                            nc.tensor.matmul(gp[:], w1t[:, kb, :],
                                             hts[q][:, kb, :],
                                             start=(kb == 0),
                                             stop=(kb == KB_ - 1))
                        for kb in range(KB_):
                            nc.tensor.matmul(up[:], w3t[:, kb, :],
                                             hts[q][:, kb, :],
                                             start=(kb == 0),
                                             stop=(kb == KB_ - 1))
                        sg = mstg.tile([128, CH], bf, tag="sg", name="sg")
                        nc.scalar.activation(sg[:], gp[:], AF.Silu)
                        last_mul = nc.vector.tensor_mul(acts[q][:, it, :],
                                                        sg[:], up[:])
                    # last-chunk norm2 sits here so its AllReduce wait
                    # cannot block the PE queue at MLP entry
                    if P[0] == 0 and it == 6:
                        norm2(NCH - 1, ps_ds2, mstg, rowp2,
                              gate=last_mul)

                # convert hts in place to h/8 = hhat * rms/8 for the
                # residual fold (g/u no longer needs the normalized value)
                for q in P:
                    rr2 = rowp2.tile([1, CH], f32, tag="rms",
                                     name=f"rr2{q}")
                    nc.sync.dma_start(rr2[:], rms_d[q].ap())
                    r8 = rowp2.tile([1, CH], bf, tag="rinv", name=f"r8{q}")
                    nc.vector.tensor_scalar_mul(r8[:], rr2[:], 1.0 / NC)
                    r8w[q] = roww2.tile([128, 4, CH], bf, tag="rcsw",
                                        name=f"r8w{q}")
                    for a in range(4):
                        nc.gpsimd.partition_broadcast(r8w[q][:, a, :],
                                                      r8[:])
                    for g in range(KB_ // 4):
                        nc.vector.tensor_mul(hts[q][:, 4 * g:4 * g + 4, :],
                                             hts[q][:, 4 * g:4 * g + 4, :],
                                             r8w[q][:])

                # down-proj; h/8 folded into the RS inputs
                for s8 in range(8):
                    if s8 in w2pre:
                        w2s = w2pre.pop(s8)
                    else:
                        w2s = mw2.tile([128, IT_, 512], bf, tag="w2s",
                                       name="w2s")
                        nc.scalar.dma_start(w2s[:], w2R.ap()[s8])
                    for q in reversed(P):
                        for hb4 in range(4):
                            hb = s8 * 4 + hb4
                            accd = ps_d.tile([128, CH], f32, tag="d",
                                             name="d")
                            for it in range(IT_):
                                nc.tensor.matmul(
                                    accd[:],
                                    w2s[:, it, hb4 * 128:(hb4 + 1) * 128],
                                    acts[q][:, it, :],
                                    start=(it == 0), stop=(it == IT_ - 1))
                            ob = mstg.tile([128, CH], bf, tag="ob",
                                           name="ob")
                            nc.vector.tensor_add(ob[:], accd[:],
                                                 hts[q][:, hb, :])
                            nc.sync.dma_start(
                                rsm_in[q][hb // 16].ap()[:, hb % 16, :],
                                ob[:])
                    if s8 == 3:
                        for q in reversed(P):
                            reducescatter(rsm_in[q][0], rsm_out[q][0])
                for q in reversed(P):
                    reducescatter(rsm_in[q][1], rsm_out[q][1])

        mph.close()

        # ===== final: out = f32(rsm_out) (h folded via h/8 trick) =====
        with ExitStack() as fph, nc.named_scope("fin"):
            finp = fph.enter_context(tc.tile_pool(name="finp", bufs=2))
            for q in (0, 1, 3, 2):
                for half in range(2):
                    rt = finp.tile([128, 2, CH], bf, tag="frt", name="frt")
                    nc.sync.dma_start(rt[:], rsm_out[q][half].ap())
                    ot = finp.tile([128, 2, CH], f32, tag="fot", name="fot")
                    nc.vector.tensor_copy(ot[:], rt[:])
                    nc.sync.dma_start(out_own.ap()[q][half], ot[:])

    nc.compile()
    return nc


def _get_compiled():
    global _compiled
    if _compiled is None:
        _compiled = _build()
    return _compiled


def _prep_inputs(inputs):
    x = np.asarray(inputs["hidden_states"], np.float32)
    pos = np.asarray(inputs["position_ids"]).astype(np.float32)
    wqkv = np.asarray(inputs["wqkv"], np.float32)
    wo = np.asarray(inputs["wo"], np.float32)
    w1 = np.asarray(inputs["w1"], np.float32)
    w3 = np.asarray(inputs["w3"], np.float32)
    w2 = np.asarray(inputs["w2"], np.float32)
    anw = np.asarray(inputs["attn_norm_w"], np.float32)
    fnw = np.asarray(inputs["ffn_norm_w"], np.float32)

    inv_freq = 1.0 / (THETA ** (np.arange(0, D, 2, dtype=np.float32) / D))
    freqs = pos[:, None] * inv_freq
    cosT_np = np.ascontiguousarray(np.cos(freqs).T.astype(bf16))
    sinT_np = np.ascontiguousarray(np.sin(freqs).T.astype(bf16))
    ident_np = np.ascontiguousarray(np.eye(128, dtype=bf16))

    # causal masks for diagonal tiles: masks[p, r, f] = (f >= 128*r + p)
    p_ = np.arange(128)[:, None, None]
    r_ = np.arange(4)[None, :, None]
    f_ = np.arange(CH)[None, None, :]
    masks_np = np.ascontiguousarray((f_ >= 128 * r_ + p_).astype(bf16))

    xT_np = np.ascontiguousarray(
        x.T.reshape(KB_, 128, T).transpose(1, 0, 2).astype(bf16))

    wqkv_f = wqkv * anw[None, :]
    w1_f = w1 * fnw[None, :]
    w3_f = w3 * fnw[None, :]

    in_maps = []
    for c in range(NC):
        qrows = np.arange(JD * c, JD * (c + 1))
        krows = H * D + np.arange(D * c, D * (c + 1))
        vrows = (H + K) * D + np.arange(D * c, D * (c + 1))
        rows = np.concatenate([qrows, krows, vrows])
        wqkvT = wqkv_f[rows].T                      # [HID, 768]
        w1T = w1_f[IS * c:IS * (c + 1)].T           # [HID, IS]
        w3T = w3_f[IS * c:IS * (c + 1)].T
        in_maps.append({
            "xT": xT_np,
            "cosT": cosT_np, "sinT": sinT_np, "ident": ident_np,
            "masksI": masks_np,
            "wqkvR": np.ascontiguousarray(
                wqkvT.reshape(KB_, 128, 6, 128).transpose(2, 1, 0, 3)
                .astype(bf16)),
            "woR": np.ascontiguousarray(
                wo[:, JD * c:JD * (c + 1)].T.reshape(QH, 128, 8, 512)
                .transpose(2, 1, 0, 3).astype(bf16)),
            "w1R": np.ascontiguousarray(
                w1T.reshape(KB_, 128, IT_, 128).transpose(2, 1, 0, 3)
                .astype(bf16)),
            "w3R": np.ascontiguousarray(
                w3T.reshape(KB_, 128, IT_, 128).transpose(2, 1, 0, 3)
                .astype(bf16)),
            "w2R": np.ascontiguousarray(
                w2[:, IS * c:IS * (c + 1)].T.reshape(IT_, 128, 8, 512)
                .transpose(2, 1, 0, 3).astype(bf16)),
        })
    return in_maps


def run(inputs, trace=False):
    """Returns (output, BassKernelResults)."""
    from concourse import bass_utils
    nc = _get_compiled()
    in_maps = _prep_inputs(inputs)
    res = bass_utils.run_bass_kernel_spmd(
        nc, in_maps, core_ids=list(range(NC)), trace=trace)
    # out_own[c][q, half, p16, kb16, t]: outT[(16*half+kb16)*128+16*c+p16,
    #                                        512*q+t]
    arr = np.stack([res.results[c]["out_own"] for c in range(NC)])
    outT = arr.transpose(2, 4, 0, 3, 1, 5).reshape(HID, T)
    return np.ascontiguousarray(outT.T), res


def kernel(**inputs):
    out, _ = run(inputs)
    return out


# revision 26
# speedup vs baseline: 1.0334x; 1.0123x over previous
"""InternLM2 decoder layer on 8 trn2 NeuronCores, tensor-parallel (bass/Tile).

Self-contained: hardcodes shapes/sharding. Host pre-transposes x (xT
replicated to all cores) and pre-tiles weights (bf16, RMSNorm gammas folded
into consuming matmul weights); the device computes the whole layer in
transposed [hid, tok] layout; the host reassembles + transposes the output.

Design notes:
- No AllGather of normalized activations. Every core holds full xT and
  computes the rmsnorm scale row (1/rms per token) itself: sum-of-squares
  via ones-vector matmul (partition reduction), sqrt+eps on ACT, reciprocal
  on DVE, partition broadcast on gpsimd, batched [128,4,CH] DVE multiplies
  (DVE per-op overhead is large; 4-ktile batching keeps DVE off the
  critical path).
- Attention output is produced TRANSPOSED (oT[hid, tok], wo stationary,
  aoT moving) and summed across cores with a per-chunk AllReduce; h and
  norm2 then happen locally on every core in the layout the MLP consumes.
  All DMA uses >=1KB per-partition runs: short strided runs measurably slow
  concurrent matmuls ~20%.
- Softmax: head pairs with score->exp->prob(pv/den) software pipelining
  (pv/den lag one key-tile behind scores) so the PE never waits on the
  scalar engine's exp. Denominator rows for the pair pack at partitions
  0/32 of one PSUM bank. PSUM->SBUF copies run on ACT, not DVE.
- MLP processes token chunks in fused pairs per weight load. After g/u,
  hts is converted in place to h/8 (hhat*rms/8) so the down-proj just adds
  it into its partial outputs; the kb-halved ReduceScatter then directly
  yields the final residual sum, and the tail is a bf16->f32 copy of the
  last RS pieces. norm2 for the last chunk is emitted inside the first MLP
  pair so its AllReduce wait does not block the PE queue.
"""
import sys
import numpy as np
import ml_dtypes

sys.path.insert(0, "/opt/trn_rl_repo")

HID, H, K, D, INTER, T = 4096, 32, 8, 128, 14336, 2048
EPS, THETA = 1e-5, 1000000.0
NC = 8                 # cores
QH = H // NC           # q heads per core = 4
JD = QH * D            # per-core attn out dim = 512
IS = INTER // NC       # inter shard = 1792
CH = 512               # token chunk
NCH = T // CH          # 4
KB_ = HID // 128       # 32 k-tiles
IT_ = IS // 128        # 14 i-tiles
SCALE = 1.0 / np.sqrt(D)

bf16 = ml_dtypes.bfloat16

_compiled = None


def _build():
    from contextlib import ExitStack
    import concourse.bacc as bacc
    import concourse.bass as bass
    import concourse.tile as tile
    from concourse import mybir

    f32 = mybir.dt.float32
    bf = mybir.dt.bfloat16
    AF = mybir.ActivationFunctionType
    PSUM = bass.MemorySpace.PSUM

    nc = bacc.Bacc("TRN2", target_bir_lowering=False, debug=False,
                   num_devices=NC)

    # ---- I/O ----
    xT = nc.dram_tensor("xT", [128, KB_, T], bf, kind="ExternalInput")
    cosT = nc.dram_tensor("cosT", [D // 2, T], bf, kind="ExternalInput")
    sinT = nc.dram_tensor("sinT", [D // 2, T], bf, kind="ExternalInput")
    ident = nc.dram_tensor("ident", [128, 128], bf, kind="ExternalInput")
    masksI = nc.dram_tensor("masksI", [128, 4, CH], bf, kind="ExternalInput")
    wqkvR = nc.dram_tensor("wqkvR", [6, 128, KB_, 128], bf,
                           kind="ExternalInput")
    woR = nc.dram_tensor("woR", [8, 128, QH, 512], bf, kind="ExternalInput")
    w1R = nc.dram_tensor("w1R", [IT_, 128, KB_, 128], bf,
                         kind="ExternalInput")
    w3R = nc.dram_tensor("w3R", [IT_, 128, KB_, 128], bf,
                         kind="ExternalInput")
    w2R = nc.dram_tensor("w2R", [8, 128, IT_, 512], bf, kind="ExternalInput")
    # out_own[q][half][p16][kb16][t]: hid row = (16*half+kb16)*128+16*core+p16
    out_own = nc.dram_tensor("out_own", [NCH, 2, 16, 16, CH], f32,
                             kind="ExternalOutput")

    # ---- internal DRAM ----
    ar_in = [nc.dram_tensor(f"ar_in{j}", [128, KB_, CH], bf, kind="Internal")
             for j in range(NCH)]
    ar_out = [nc.dram_tensor(f"ar_out{j}", [128, KB_, CH], bf,
                             kind="Internal", addr_space="Shared")
              for j in range(NCH)]
    hhat = [nc.dram_tensor(f"hhat{j}", [128, KB_, CH], bf, kind="Internal")
            for j in range(NCH)]
    rms_d = [nc.dram_tensor(f"rms_d{j}", [1, CH], f32, kind="Internal")
             for j in range(NCH)]
    rsm_in = [[nc.dram_tensor(f"rsm_in{q}_{h}", [128, 16, CH], bf,
                              kind="Internal") for h in range(2)]
              for q in range(NCH)]
    rsm_out = [[nc.dram_tensor(f"rsm_out{q}_{h}", [16, 16, CH], bf,
                               kind="Internal")
                for h in range(2)] for q in range(NCH)]

    RG = [list(range(NC))]

    def allreduce(in_t, out_t):
        nc.gpsimd.collective_compute(
            "AllReduce", mybir.AluOpType.add, replica_groups=RG,
            ins=[in_t.ap()], outs=[out_t.ap()])

    def reducescatter(in_t, out_t):
        nc.gpsimd.collective_compute(
            "ReduceScatter", mybir.AluOpType.add, replica_groups=RG,
            ins=[in_t.ap()], outs=[out_t.ap()])

    with tile.TileContext(nc) as tc, ExitStack() as top:
        const = top.enter_context(tc.tile_pool(name="const", bufs=1))
        ones_sb = const.tile([128, 1], bf)
        nc.vector.memset(ones_sb[:], 1.0)
        eps_sb = const.tile([1, 1], f32)
        nc.vector.memset(eps_sb[:], EPS)

        # strip-wise h = xT + attn (ar_out[j]); spill RAW h + rms row;
        # the MLP normalizes chunks on load.
        def norm2(j, dspool, sqpool, rpool, gate=None):
            def _gate(inst):
                if gate is not None:
                    bass._add_dep_helper(
                        inst.ins, gate.ins, sync=False,
                        reason="norm2 scheduled after softmax")
            with nc.named_scope(f"norm2_{j}"):
                ds = dspool.tile([65, CH], f32, tag="ds", name=f"h{j}ds")
                for g in range(8):
                    hs = sqpool.tile([128, 4, CH], bf, tag="xs",
                                     name=f"h{j}hs")
                    _gate(nc.sync.dma_start(
                        hs[:], ar_out[j].ap()[:, 4 * g:4 * g + 4, :]))
                    xs = sqpool.tile([128, 4, CH], bf, tag="sq",
                                     name=f"h{j}xs")
                    _gate(nc.sync.dma_start(
                        xs[:], xT.ap()[:, 4 * g:4 * g + 4,
                                       j * CH:(j + 1) * CH]))
                    nc.vector.tensor_add(hs[:], hs[:], xs[:])
                    nc.sync.dma_start(
                        hhat[j].ap()[:, 4 * g:4 * g + 4, :], hs[:])
                    nc.vector.tensor_mul(xs[:], hs[:], hs[:])
                    for a in range(4):
                        kb = 4 * g + a
                        nc.tensor.matmul(ds[64:65, :], ones_sb[:],
                                         xs[:, a, :], start=(kb == 0),
                                         stop=(kb == KB_ - 1))
                rms = rpool.tile([1, CH], f32, tag="rms", name=f"h{j}rms")
                nc.scalar.activation(rms[:], ds[64:65, :], AF.Sqrt,
                                     scale=1.0 / HID, bias=eps_sb[:])
                nc.sync.dma_start(rms_d[j].ap(), rms[:])

        # ======================= attention phase =======================
        aph = ExitStack()
        aconst = aph.enter_context(tc.tile_pool(name="aconst", bufs=1))
        wqp = aph.enter_context(tc.tile_pool(name="wqkv", bufs=1))
        wop = aph.enter_context(tc.tile_pool(name="wo", bufs=2))
        xpool = aph.enter_context(tc.tile_pool(name="xTp", bufs=2))
        kvp = aph.enter_context(tc.tile_pool(name="kv", bufs=1))
        qa = aph.enter_context(tc.tile_pool(name="qa", bufs=2))
        aop = aph.enter_context(tc.tile_pool(name="aop", bufs=1))
        rp = aph.enter_context(tc.tile_pool(name="ropev", bufs=2))
        rpt = aph.enter_context(tc.tile_pool(name="ropet", bufs=1))
        ptp = aph.enter_context(tc.tile_pool(name="probs", bufs=5))
        rowp = aph.enter_context(tc.tile_pool(name="rows", bufs=2))
        stg = aph.enter_context(tc.tile_pool(name="stage", bufs=2))
        ps_acc = aph.enter_context(tc.tile_pool(name="accps", bufs=2,
                                                space=PSUM))
        ps_sc = aph.enter_context(tc.tile_pool(name="scps", bufs=3,
                                               space=PSUM))
        ps_pv = aph.enter_context(tc.tile_pool(name="pvps", bufs=2,
                                               space=PSUM))
        ps_ds = aph.enter_context(tc.tile_pool(name="dsps", bufs=1,
                                               space=PSUM))

        ident_sb = aconst.tile([128, 128], bf)
        masks_sb = aconst.tile([128, 4, CH], bf)
        cos_sb = aconst.tile([D // 2, T], bf)
        sin_sb = aconst.tile([D // 2, T], bf)

        kT_sb = kvp.tile([128, T], bf)              # roped K, [d, s]
        v_sb = kvp.tile([128, T // 128, D], bf)     # [s-part, s-tile, d]

        def rope(dst, src, c, s):
            t1 = rpt.tile([64, CH], f32, tag="rp1")
            t2 = rpt.tile([64, CH], f32, tag="rp2")
            nc.vector.tensor_mul(t1[:], src[0:64, :], c)
            nc.vector.tensor_mul(t2[:], src[64:128, :], s)
            nc.vector.tensor_sub(dst[0:64, :], t1[:], t2[:])
            nc.vector.tensor_mul(t1[:], src[64:128, :], c)
            nc.vector.tensor_mul(t2[:], src[0:64, :], s)
            return nc.vector.tensor_add(dst[64:128, :], t1[:], t2[:])

        def xhat_prep(j, xt=None):
            # raw x chunk; rinv row folded into per-chunk cosR/sinR (rope
            # is linear in cos/sin) and an rcs tile for V.
            if xt is None:
                xt = xpool.tile([128, KB_, CH], bf, tag="xt", name=f"xt{j}")
                for g_ in range(8):
                    nc.sync.dma_start(
                        xt[:, 4 * g_:4 * g_ + 4, :],
                        xT.ap()[:, 4 * g_:4 * g_ + 4,
                                j * CH:(j + 1) * CH])
            ds = ps_ds.tile([65, CH], f32, tag="ds", name=f"x{j}ds")
            for g in range(KB_ // 4):
                sq = stg.tile([128, 4, CH], bf, tag="sq", name=f"x{j}sq")
                nc.vector.tensor_mul(sq[:], xt[:, 4 * g:4 * g + 4, :],
                                     xt[:, 4 * g:4 * g + 4, :])
                for a in range(4):
                    kb = 4 * g + a
                    nc.tensor.matmul(ds[64:65, :], ones_sb[:], sq[:, a, :],
                                     start=(kb == 0), stop=(kb == KB_ - 1))
            rms = rowp.tile([1, CH], f32, tag="rms", name=f"x{j}rms")
            nc.scalar.activation(rms[:], ds[64:65, :], AF.Sqrt,
                                 scale=1.0 / HID, bias=eps_sb[:])
            rin0 = rowp.tile([1, CH], f32, tag="rin0", name=f"x{j}r0")
            nc.vector.reciprocal_approx_fast(rin0[:], rms[:])
            rinv = rowp.tile([1, CH], bf, tag="rinv", name=f"x{j}ri")
            nc.vector.tensor_copy(rinv[:], rin0[:])
            rcs = rowp.tile([128, CH], bf, tag="rcs", name=f"x{j}rcs")
            nc.gpsimd.partition_broadcast(rcs[:], rinv[:])
            t0_ = j * CH
            cosR = rp.tile([64, CH], bf, tag="cosR", name=f"cosR{j}")
            sinR = rp.tile([64, CH], bf, tag="sinR", name=f"sinR{j}")
            nc.vector.tensor_mul(cosR[:], cos_sb[:, t0_:t0_ + CH],
                                 rcs[0:64, :])
            nc.vector.tensor_mul(sinR[:], sin_sb[:, t0_:t0_ + CH],
                                 rcs[0:64, :])
            return xt, cosR, sinR, rcs

        wq_sb = wqp.tile([128, 6, KB_, 128], bf)
        for m in range(6):
            nc.scalar.dma_start(wq_sb[:, m, :, :], wqkvR.ap()[m])

        xt00 = xpool.tile([128, KB_, CH], bf, tag="xt", name="xt0pre")
        for g_ in range(8):
            nc.sync.dma_start(xt00[:, 4 * g_:4 * g_ + 4, :],
                              xT.ap()[:, 4 * g_:4 * g_ + 4, 0:CH])
        nc.sync.dma_start(cos_sb[:], cosT.ap())
        nc.sync.dma_start(sin_sb[:], sinT.ap())
        nc.sync.dma_start(ident_sb[:], ident.ap())
        nc.sync.dma_start(masks_sb[:], masksI.ap())

        xts = [None] * NCH
        xts[0] = xhat_prep(0, xt00)
        for j in range(NCH):
            t0 = j * CH
            ns = 4 * j + 4
            with nc.named_scope(f"attn{j}"):
                xt, cosR, sinR, rcs = xts[j]
                # ---- QKV (resident weights) ----
                qT = qa.tile([128, QH, CH], bf, tag="qT")
                for m in range(6):
                    acc = ps_acc.tile([128, CH], f32, tag="acc", name="acc")
                    for kb in range(KB_):
                        nc.tensor.matmul(acc[:], wq_sb[:, m, kb, :],
                                         xt[:, kb, :],
                                         start=(kb == 0),
                                         stop=(kb == KB_ - 1))
                    if m < QH:
                        rope(qT[:, m, :], acc, cosR[:], sinR[:])
                    elif m == QH:
                        rope(kT_sb[:, t0:t0 + CH], acc, cosR[:], sinR[:])
                    else:
                        vb = rp.tile([128, CH], bf, tag="vb")
                        nc.vector.tensor_mul(vb[:], acc[:], rcs[:])
                        for sb_ in range(CH // 128):
                            tp = ps_sc.tile([128, 128], bf, tag="sc",
                                            name="tp")
                            nc.tensor.transpose(
                                tp[:], vb[:, sb_ * 128:(sb_ + 1) * 128],
                                ident_sb[:])
                            nc.vector.tensor_copy(v_sb[:, 4 * j + sb_, :],
                                                  tp[:])
                # prep next chunk's x while softmax runs
                if j + 1 < NCH:
                    xts[j + 1] = xhat_prep(j + 1)

                # ---- softmax: head pairs, pv/den lag one step ----
                aoT = aop.tile([128, QH, CH], bf, tag="aoT")
                for pr in range(QH // 2):
                    h0, h1 = 2 * pr, 2 * pr + 1
                    ds = ps_ds.tile([65, CH], f32, tag="ds",
                                    name=f"den{j}_{pr}")
                    pvs = {}
                    doff = {h0: 0, h1: 32}
                    for h in (h0, h1):
                        pvs[h] = ps_pv.tile([128, CH], f32, tag="pv",
                                            name=f"pv{h}")

                    def emit_pd(h, si, pT):
                        nc.tensor.matmul(pvs[h][:], v_sb[:, si, :], pT[:],
                                         start=(si == 0),
                                         stop=(si == ns - 1))
                        o = doff[h]
                        nc.tensor.matmul(ds[o:o + 1, :], ones_sb[:], pT[:],
                                         start=(si == 0),
                                         stop=(si == ns - 1))

                    pend = []
                    for si in range(ns):
                        for h in (h0, h1):
                            sc = ps_sc.tile([128, CH], f32, tag="sc",
                                            name="sc")
                            nc.tensor.matmul(
                                sc[:], kT_sb[:, si * 128:(si + 1) * 128],
                                qT[:, h, :], start=True, stop=True)
                            pT = ptp.tile([128, CH], bf, tag="pT", name="pT")
                            last_exp = nc.scalar.activation(
                                pT[:], sc[:], AF.Exp, scale=SCALE)
                            if si >= 4 * j:      # diagonal: zero s > t
                                pm = ptp.tile([128, CH], bf, tag="pm",
                                              name="pm")
                                nc.vector.tensor_mul(
                                    pm[:], pT[:],
                                    masks_sb[:, si - 4 * j, :])
                                pT = pm
                            pend.append((h, si, pT))
                        while len(pend) > 4:     # pv/den lag two key-tiles
                            emit_pd(*pend.pop(0))
                    for tup in pend:
                        emit_pd(*tup)
                    for h in (h0, h1):
                        o = doff[h]
                        dsb = rowp.tile([1, CH], f32, tag="rms",
                                        name="dsb")
                        nc.vector.tensor_copy(dsb[:], ds[o:o + 1, :])
                        rr0 = rowp.tile([1, CH], f32, tag="rin0",
                                        name="rr0")
                        nc.vector.reciprocal_approx_fast(rr0[:], dsb[:])
                        rcs = rowp.tile([128, CH], f32, tag="rcs",
                                        name="rcs")
                        nc.gpsimd.partition_broadcast(rcs[:], rr0[:])
                        nc.vector.tensor_mul(aoT[:, h, :], pvs[h][:],
                                             rcs[:])

                # ---- wo -> oT[hid, tok] partials -> AllReduce ----
                for g in range(8):
                    woS = wop.tile([128, QH, 512], bf, tag="woS", name="woS")
                    nc.scalar.dma_start(woS[:], woR.ap()[g])
                    for hb4 in range(4):
                        acc = ps_acc.tile([128, CH], f32, tag="acc",
                                          name="woacc")
                        for kb in range(QH):
                            nc.tensor.matmul(
                                acc[:],
                                woS[:, kb, hb4 * 128:(hb4 + 1) * 128],
                                aoT[:, kb, :],
                                start=(kb == 0), stop=(kb == QH - 1))
                        ob = stg.tile([128, CH], bf, tag="ob", name="ob")
                        nc.vector.tensor_copy(ob[:], acc[:])
                        nc.sync.dma_start(
                            ar_in[j].ap()[:, g * 4 + hb4, :], ob[:])
                allreduce(ar_in[j], ar_out[j])
                # previous chunk's norm2 AFTER this chunk's softmax/wo so a
                # slow AllReduce can never head-of-line-block the PE queue
                if j >= 1:
                    norm2(j - 1, ps_ds, stg, rowp, gate=last_exp)

        aph.close()

        # ========================= MLP phase =========================
        mph = ExitStack()
        hpool = mph.enter_context(tc.tile_pool(name="hh", bufs=2))
        actp = mph.enter_context(tc.tile_pool(name="act", bufs=1))
        mw = mph.enter_context(tc.tile_pool(name="mw", bufs=2))
        mw2 = mph.enter_context(tc.tile_pool(name="mw2", bufs=2))
        mstg = mph.enter_context(tc.tile_pool(name="mstage", bufs=2))
        rowp2 = mph.enter_context(tc.tile_pool(name="rows2", bufs=1))
        roww2 = mph.enter_context(tc.tile_pool(name="roww2", bufs=2))
        ps_g = mph.enter_context(tc.tile_pool(name="gups", bufs=2,
                                              space=PSUM))
        ps_d = mph.enter_context(tc.tile_pool(name="dps", bufs=2,
                                              space=PSUM))
        ps_ds2 = mph.enter_context(tc.tile_pool(name="dsps2", bufs=1,
                                                space=PSUM))

        for P in ((0, 1), (2, 3)):
            with nc.named_scope(f"mlp{P[0]}{P[1]}"):
                hts, r8w, w2pre = {}, {}, {}
                for q in P:
                    hts[q] = hpool.tile([128, KB_, CH], bf, tag="hh",
                                        name=f"hh{q}")
                    nc.sync.dma_start(hts[q][:], hhat[q].ap())
                    rr3 = rowp2.tile([1, CH], f32, tag="rms",
                                     name=f"rr3{q}")
                    nc.sync.dma_start(rr3[:], rms_d[q].ap())
                    ri30 = rowp2.tile([1, CH], f32, tag="rin0",
                                      name=f"ri30{q}")
                    nc.vector.reciprocal_approx_fast(ri30[:], rr3[:])
                    ri3 = rowp2.tile([1, CH], bf, tag="rinv",
                                     name=f"ri3{q}")
                    nc.vector.tensor_copy(ri3[:], ri30[:])
                    rw3 = roww2.tile([128, 4, CH], bf, tag="rcsw",
                                     name=f"rw3{q}")
                    for a in range(4):
                        nc.gpsimd.partition_broadcast(rw3[:, a, :],
                                                      ri3[:])
                    for g in range(KB_ // 4):
                        nc.vector.tensor_mul(
                            hts[q][:, 4 * g:4 * g + 4, :],
                            hts[q][:, 4 * g:4 * g + 4, :], rw3[:])
                acts = {q: actp.tile([128, IT_, CH], bf, tag=f"act{qi}",
                                     name=f"act{q}")
                        for qi, q in enumerate(P)}
                for it in range(IT_):
                    w1t = mw.tile([128, KB_, 128], bf, tag="w1t",
                                  name="w1t")
                    w3t = mw.tile([128, KB_, 128], bf, tag="w3t",
                                  name="w3t")
                    nc.scalar.dma_start(w1t[:], w1R.ap()[it])
                    nc.scalar.dma_start(w3t[:], w3R.ap()[it])
                    for q in P:
                        gp = ps_g.tile([128, CH], f32, tag="g", name="g")
                        up = ps_g.tile([128, CH], f32, tag="u", name="u")
                        for kb in range(KB_):
                            nc.tensor.matmul(gp[:], w1t[:, kb, :],
                                             hts[q][:, kb, :],
                                             start=(kb == 0),
                                             stop=(kb == KB_ - 1))
                        for kb in range(KB_):
                            nc.tensor.matmul(up[:], w3t[:, kb, :],
                                             hts[q][:, kb, :],
                                             start=(kb == 0),
                                             stop=(kb == KB_ - 1))
                        sg = mstg.tile([128, CH], bf, tag="sg", name="sg")
                        nc.scalar.activation(sg[:], gp[:], AF.Silu)
                        last_mul = nc.vector.tensor_mul(acts[q][:, it, :],
                                                        sg[:], up[:])
                    # last-chunk norm2 sits here so its AllReduce wait
                    # cannot block the PE queue at MLP entry
                    if P[0] == 0 and it == 6:
                        norm2(NCH - 1, ps_ds2, mstg, rowp2,
                              gate=last_mul)

                # convert hts in place to h/8 = hhat * rms/8 for the
                # residual fold (g/u no longer needs the normalized value)
                for q in P:
                    rr2 = rowp2.tile([1, CH], f32, tag="rms",
                                     name=f"rr2{q}")
                    nc.sync.dma_start(rr2[:], rms_d[q].ap())
                    r8 = rowp2.tile([1, CH], bf, tag="rinv", name=f"r8{q}")
                    nc.vector.tensor_scalar_mul(r8[:], rr2[:], 1.0 / NC)
                    r8w[q] = roww2.tile([128, 4, CH], bf, tag="rcsw",
                                        name=f"r8w{q}")
                    for a in range(4):
                        nc.gpsimd.partition_broadcast(r8w[q][:, a, :],
                                                      r8[:])
                    for g in range(KB_ // 4):
                        nc.vector.tensor_mul(hts[q][:, 4 * g:4 * g + 4, :],
                                             hts[q][:, 4 * g:4 * g + 4, :],
                                             r8w[q][:])

                # down-proj; h/8 folded into the RS inputs
                for s8 in range(8):
                    if s8 in w2pre:
                        w2s = w2pre.pop(s8)
                    else:
                        w2s = mw2.tile([128, IT_, 512], bf, tag="w2s",
                                       name="w2s")
                        nc.sync.dma_start(w2s[:], w2R.ap()[s8])
                    for q in reversed(P):
                        for hb4 in range(4):
                            hb = s8 * 4 + hb4
                            accd = ps_d.tile([128, CH], f32, tag="d",
                                             name="d")
                            for it in range(IT_):
                                nc.tensor.matmul(
                                    accd[:],
                                    w2s[:, it, hb4 * 128:(hb4 + 1) * 128],
                                    acts[q][:, it, :],
                                    start=(it == 0), stop=(it == IT_ - 1))
                            ob = mstg.tile([128, CH], bf, tag="ob",
                                           name="ob")
                            nc.vector.tensor_add(ob[:], accd[:],
                                                 hts[q][:, hb, :])
                            nc.sync.dma_start(
                                rsm_in[q][hb // 16].ap()[:, hb % 16, :],
                                ob[:])
                    if s8 == 3:
                        for q in reversed(P):
                            reducescatter(rsm_in[q][0], rsm_out[q][0])
                for q in reversed(P):
                    reducescatter(rsm_in[q][1], rsm_out[q][1])

        mph.close()

        # ===== final: out = f32(rsm_out) (h folded via h/8 trick) =====
        with ExitStack() as fph, nc.named_scope("fin"):
            finp = fph.enter_context(tc.tile_pool(name="finp", bufs=2))
            for q in (0, 1, 3, 2):
                for half in range(2):
                    rt = finp.tile([128, 2, CH], bf, tag="frt", name="frt")
                    nc.sync.dma_start(rt[:], rsm_out[q][half].ap())
                    ot = finp.tile([128, 2, CH], f32, tag="fot", name="fot")
                    nc.vector.tensor_copy(ot[:], rt[:])
                    nc.sync.dma_start(out_own.ap()[q][half], ot[:])

    nc.compile()
    return nc


def _get_compiled():
    global _compiled
    if _compiled is None:
        _compiled = _build()
    return _compiled


def _prep_inputs(inputs):
    x = np.asarray(inputs["hidden_states"], np.float32)
    pos = np.asarray(inputs["position_ids"]).astype(np.float32)
    wqkv = np.asarray(inputs["wqkv"], np.float32)
    wo = np.asarray(inputs["wo"], np.float32)
    w1 = np.asarray(inputs["w1"], np.float32)
    w3 = np.asarray(inputs["w3"], np.float32)
    w2 = np.asarray(inputs["w2"], np.float32)
    anw = np.asarray(inputs["attn_norm_w"], np.float32)
    fnw = np.asarray(inputs["ffn_norm_w"], np.float32)

    inv_freq = 1.0 / (THETA ** (np.arange(0, D, 2, dtype=np.float32) / D))
    freqs = pos[:, None] * inv_freq
    cosT_np = np.ascontiguousarray(np.cos(freqs).T.astype(bf16))
    sinT_np = np.ascontiguousarray(np.sin(freqs).T.astype(bf16))
    ident_np = np.ascontiguousarray(np.eye(128, dtype=bf16))

    # causal masks for diagonal tiles: masks[p, r, f] = (f >= 128*r + p)
    p_ = np.arange(128)[:, None, None]
    r_ = np.arange(4)[None, :, None]
    f_ = np.arange(CH)[None, None, :]
    masks_np = np.ascontiguousarray((f_ >= 128 * r_ + p_).astype(bf16))

    xT_np = np.ascontiguousarray(
        x.T.reshape(KB_, 128, T).transpose(1, 0, 2).astype(bf16))

    wqkv_f = wqkv * anw[None, :]
    w1_f = w1 * fnw[None, :]
    w3_f = w3 * fnw[None, :]

    in_maps = []
    for c in range(NC):
        qrows = np.arange(JD * c, JD * (c + 1))
        krows = H * D + np.arange(D * c, D * (c + 1))
        vrows = (H + K) * D + np.arange(D * c, D * (c + 1))
        rows = np.concatenate([qrows, krows, vrows])
        wqkvT = wqkv_f[rows].T                      # [HID, 768]
        w1T = w1_f[IS * c:IS * (c + 1)].T           # [HID, IS]
        w3T = w3_f[IS * c:IS * (c + 1)].T
        in_maps.append({
            "xT": xT_np,
            "cosT": cosT_np, "sinT": sinT_np, "ident": ident_np,
            "masksI": masks_np,
            "wqkvR": np.ascontiguousarray(
                wqkvT.reshape(KB_, 128, 6, 128).transpose(2, 1, 0, 3)
                .astype(bf16)),
            "woR": np.ascontiguousarray(
                wo[:, JD * c:JD * (c + 1)].T.reshape(QH, 128, 8, 512)
                .transpose(2, 1, 0, 3).astype(bf16)),
            "w1R": np.ascontiguousarray(
                w1T.reshape(KB_, 128, IT_, 128).transpose(2, 1, 0, 3)
                .astype(bf16)),
            "w3R": np.ascontiguousarray(
                w3T.reshape(KB_, 128, IT_, 128).transpose(2, 1, 0, 3)
                .astype(bf16)),
            "w2R": np.ascontiguousarray(
                w2[:, IS * c:IS * (c + 1)].T.reshape(IT_, 128, 8, 512)
                .transpose(2, 1, 0, 3).astype(bf16)),
        })
    return in_maps


def run(inputs, trace=False):
    """Returns (output, BassKernelResults)."""
    from concourse import bass_utils
    nc = _get_compiled()
    in_maps = _prep_inputs(inputs)
    res = bass_utils.run_bass_kernel_spmd(
        nc, in_maps, core_ids=list(range(NC)), trace=trace)
    # out_own[c][q, half, p16, kb16, t]: outT[(16*half+kb16)*128+16*c+p16,
    #                                        512*q+t]
    arr = np.stack([res.results[c]["out_own"] for c in range(NC)])
    outT = arr.transpose(2, 4, 0, 3, 1, 5).reshape(HID, T)
    return np.ascontiguousarray(outT.T), res


def kernel(**inputs):
    out, _ = run(inputs)
    return out
